# revision 25
# baseline (speedup 1.0000x reference)
"""nn_CGBlock Trainium2 kernel: grouped channel softmax-attention branch +
grouped top-k branch, softmax-mixed, for x [16, 256, 128, 128] f32.

Data-parallel over batch: 8 NeuronCores x 2 batches each.

Raw-Bass implementation (explicit semaphores; the Tile scheduler emits
multi-wait instructions that this walrus build cannot encode - it allows
only one sync wait per instruction, so every cross-engine dependency here
is a standalone single-wait `wait_ge`).

Per-core pipeline over h-blocks of HBLK=4 rows (one tile = one h row =
128 pixels on SBUF partitions after transpose):
  SP   : channel-major HBM loads x0/x1 [128ch_half, 512 pix], stores.
  ACT  : e = exp(x); all PSUM->SBUF copies (x^T, s/num, z^T).
  Pool : xe = x*e; y = num/s (GPSIMD cannot touch PSUM, hence the sn copy).
  PE   : per-tile transposes x -> pixel-major; tiny matmuls vs constant
         masks give per-(pixel,group) s = sum_c e, num = sum_c x*e*w1;
         transpose z = [y | top8] back to z-major; delta = W2eff @ z with
         both second 1x1 convs, top_w1, and softmax(r) mixing folded in.
  DVE  : hardware top-8 `max` per (pixel,group) 32-channel window (exact
         descending-sort semantics incl. duplicates); out = x + delta.

Software pipelining: block i's tail (z transpose, delta, adds, stores) is
interleaved with block i+1's head on each engine's instruction stream.
"""

from contextlib import ExitStack

import numpy as np

import concourse.bass as bass
import concourse.mybir as mybir
from concourse.bass_utils import run_bass_kernel_spmd

F32 = mybir.dt.float32
G = 8
K = 4
ZDIM = 72  # 8 y + 8 groups * 8 max-slots

NCORES = 8
B, C, H, W = 16, 256, 128, 128
NB = B // NCORES  # batches per core

_DELTA_DT = mybir.dt.float16
_DELTA_NP = np.float16
_HBLK = 4


def _build_consts(soft_w1, soft_w2, top_w1, top_w2, r):
    soft_w1 = np.asarray(soft_w1, np.float32)
    soft_w2 = np.asarray(soft_w2, np.float32)
    top_w1 = np.asarray(top_w1, np.float32)
    top_w2 = np.asarray(top_w2, np.float32)
    r = np.asarray(r, np.float32)

    w = np.exp(r - r.max())
    w = w / w.sum()
    rt, rs = np.float32(w[0]), np.float32(w[1])

    w2eff = np.zeros((2, ZDIM, C // 2), np.float32)
    for g in range(G):
        for hf in range(2):
            cols = slice(hf * (C // 2), (hf + 1) * (C // 2))
            w2eff[hf, g, :] = rs * soft_w2[cols, g]
            for k in range(K):
                w2eff[hf, 8 + 8 * g + k, :] = rt * top_w2[cols, g] * top_w1[g, k]
    w2eff = np.ascontiguousarray(w2eff.astype(_DELTA_NP))

    masks = np.zeros((2, 128, 8), np.float32)
    for hf in range(2):
        for j in range(4):
            rows = slice(j * 32, (j + 1) * 32)
            masks[hf, rows, j] = 1.0
            masks[hf, rows, 4 + j] = soft_w1[hf * 4 + j, :]

    ident = np.eye(128, dtype=np.float32)
    return {"w2eff": w2eff, "masks": masks, "ident": ident}


def _build_kernel(NB=NB, NH=H, HBLK=_HBLK, delta_dtype=_DELTA_DT):
    assert NH % HBLK == 0 and HBLK == 4
    nc = bass.Bass("TRN2", target_bir_lowering=False, debug=False)

    x_d = nc.dram_tensor("x", [NB, C, NH, W], F32, kind="ExternalInput").ap()
    w2eff_d = nc.dram_tensor("w2eff", [2, ZDIM, 128], delta_dtype,
                             kind="ExternalInput").ap()
    masks_d = nc.dram_tensor("masks", [2, 128, 8], F32, kind="ExternalInput").ap()
    ident_d = nc.dram_tensor("ident", [128, 128], F32, kind="ExternalInput").ap()
    out_d = nc.dram_tensor("out", [NB, C, NH, W], F32, kind="ExternalOutput").ap()

    P = HBLK * 128          # 512 pixels per h-block
    NBLK = NB * (NH // HBLK)
    Exp = mybir.ActivationFunctionType.Exp

    def blk(i):
        b = i // (NH // HBLK)
        h0 = (i % (NH // HBLK)) * HBLK
        return b, h0

    with ExitStack() as ctx:
        def sb(name, shape, dtype=F32):
            return ctx.enter_context(nc.sbuf_tensor(name, shape, dtype))

        def ps(name, shape, dtype=F32):
            return ctx.enter_context(nc.psum_tensor(name, shape, dtype))

        def sem(name):
            return ctx.enter_context(nc.semaphore(name))

        # constants
        ident = sb("identc", [128, 128])
        mask0 = sb("mask0", [128, 8])
        mask1 = sb("mask1", [128, 8])
        w2e0 = sb("w2e0", [ZDIM, 128], delta_dtype)
        w2e1 = sb("w2e1", [ZDIM, 128], delta_dtype)

        # ring buffers
        XD, ED, SD, OD = 6, 3, 3, 3
        x0 = [sb(f"x0_{j}", [128, P]) for j in range(XD)]
        x1 = [sb(f"x1_{j}", [128, P]) for j in range(XD)]
        e0 = [sb(f"e0_{j}", [128, P]) for j in range(ED)]
        e1 = [sb(f"e1_{j}", [128, P]) for j in range(ED)]
        xe0 = [sb(f"xe0_{j}", [128, P]) for j in range(ED)]
        xe1 = [sb(f"xe1_{j}", [128, P]) for j in range(ED)]
        xp_sb = [[sb(f"xp_{u}_{j}", [128, 512]) for j in range(2)]
                 for u in range(2)]
        z4 = [sb(f"z4_{j}", [128, HBLK * ZDIM]) for j in range(2)]
        sn_sb = [sb(f"sn_{j}", [128, HBLK * 16]) for j in range(SD)]
        rcp = [sb(f"rcp_{j}", [128, HBLK * 8]) for j in range(SD)]
        zT_sb = [sb(f"zT_{j}", [ZDIM, P], delta_dtype) for j in range(2)]
        o0 = [sb(f"o0_{j}", [128, P]) for j in range(OD)]
        o1 = [sb(f"o1_{j}", [128, P]) for j in range(OD)]

        # psum (8 banks total)
        xp_ps = [ps(f"xpps_{u}", [128, 512]) for u in range(2)]
        sn_ps = ps("snps", [128, HBLK * 16])
        zT_ps = ps("ztps", [ZDIM, P])
        d0_ps = [ps(f"d0ps_{j}", [128, P]) for j in range(2)]
        d1_ps = [ps(f"d1ps_{j}", [128, P]) for j in range(2)]

        # semaphores
        s_x0 = [sem(f"s_x0_{j}") for j in range(XD)]
        s_x1 = [sem(f"s_x1_{j}") for j in range(XD)]
        s_st0 = [sem(f"s_st0_{j}") for j in range(OD)]
        s_st1 = [sem(f"s_st1_{j}") for j in range(OD)]
        s_cst = sem("s_cst")
        s_exp = sem("s_exp")   # +1 after exp1(i)          -> i+1
        s_xe = sem("s_xe")     # +1 after xe1(i)           -> i+1
        s_xpc = sem("s_xpc")   # +1 after xp-copy(u,i)     -> 2i+u+1
        s_snc = sem("s_snc")   # +1 after sn-copy(i)       -> i+1
        s_ztc = sem("s_ztc")   # +1 after zt-copy(i)       -> i+1
        s_div = sem("s_div")   # +1 after y-mul(i)         -> i+1
        s_rcp = sem("s_rcp")   # +1 after recip(i)         -> i+1
        s_tx = sem("s_tx")     # +1 after T(x) pair-u(i)   -> 2i+u+1
        s_mm = sem("s_mm")     # +1 after mm_sn(i) last    -> i+1
        s_tz = sem("s_tz")     # +1 after T(z)(i) last     -> i+1
        s_dl = sem("s_dl")     # +1 after delta1(i)        -> i+1
        s_mx = sem("s_mx")     # +1 after last max(i)      -> i+1
        s_ad = sem("s_ad")     # +1 after add1(i)          -> i+1

        with nc.Block() as block:

            @block.sync
            def _(sync):
                # constants: one queue (SP hwdge), FIFO
                sync.dma_start(ident[:], ident_d[:]).then_inc(s_cst, 16)
                sync.dma_start(mask0[:], masks_d[0]).then_inc(s_cst, 16)
                sync.dma_start(mask1[:], masks_d[1]).then_inc(s_cst, 16)
                sync.dma_start(w2e0[:], w2eff_d[0]).then_inc(s_cst, 16)
                sync.dma_start(w2e1[:], w2eff_d[1]).then_inc(s_cst, 16)
                for i in range(NBLK + 1):
                    if i < NBLK:
                        b, h0 = blk(i)
                        if i >= XD:
                            sync.wait_ge(s_ad, i - XD + 1)
                        j = i % XD
                        sync.dma_start(
                            x0[j].ap().rearrange("p (h w) -> p h w", h=HBLK),
                            x_d[b, 0:128, h0:h0 + HBLK, :],
                        ).then_inc(s_x0[j], 16)
                        sync.dma_start(
                            x1[j].ap().rearrange("p (h w) -> p h w", h=HBLK),
                            x_d[b, 128:256, h0:h0 + HBLK, :],
                        ).then_inc(s_x1[j], 16)


            @block.scalar
            def _(scalar):
                for i in range(NBLK + 1):
                    j = i % 2
                    if i < NBLK:
                        je = i % ED
                        if i >= ED:
                            scalar.wait_ge(s_xe, i - ED + 1)   # e WAR vs Pool
                            scalar.wait_ge(s_mm, i - ED + 1)   # e WAR vs PE
                        scalar.wait_ge(s_x0[i % XD], 16 * (i // XD + 1))
                        scalar.activation(e0[je][:], x0[i % XD][:], Exp)
                        scalar.wait_ge(s_x1[i % XD], 16 * (i // XD + 1))
                        scalar.activation(e1[je][:], x1[i % XD][:], Exp) \
                            .then_inc(s_exp, 1)
                        if i >= 2:
                            scalar.wait_ge(s_mx, i - 1)   # xp_sb WAR vs maxes
                        for u in range(2):
                            scalar.wait_ge(s_tx, 2 * i + u + 1)
                            scalar.copy(xp_sb[u][j][:], xp_ps[u][:]) \
                                .then_inc(s_xpc, 1)
                    if i >= 1:
                        if i >= 3:
                            scalar.wait_ge(s_dl, i - 2)   # zT_sb WAR vs delta
                        scalar.wait_ge(s_tz, i)
                        scalar.copy(zT_sb[(i - 1) % 2][:], zT_ps[:]) \
                            .then_inc(s_ztc, 1)
                    if i < NBLK:
                        if i >= SD:
                            scalar.wait_ge(s_div, i - SD + 1)  # sn_sb WAR
                        scalar.wait_ge(s_mm, i + 1)
                        scalar.copy(sn_sb[i % SD][:], sn_ps[:]) \
                            .then_inc(s_snc, 1)

            @block.gpsimd
            def _(gpsimd):
                for i in range(NBLK + 2):
                    j = i % 2
                    if 1 <= i <= NBLK:
                        # y(i-1) = num(i-1) * (1/s(i-1))
                        jp = (i - 1) % 2
                        js = (i - 1) % SD
                        if i >= 3:
                            gpsimd.wait_ge(s_tz, i - 2)   # z4 WAR vs T(z)
                        gpsimd.wait_ge(s_rcp, i)
                        sn4 = sn_sb[js].ap().rearrange(
                            "p (t hf x g) -> p t hf x g", t=HBLK, hf=2, x=2)
                        rc4 = rcp[js].ap().rearrange(
                            "p (t hf g) -> p t hf g", t=HBLK, hf=2)
                        z4v = z4[jp].ap().rearrange(
                            "p (t a hf g) -> p t a hf g", t=HBLK, a=9, hf=2)
                        gpsimd.tensor_tensor(
                            z4v[:, :, 0, :, :], sn4[:, :, :, 1, :],
                            rc4, op=mybir.AluOpType.mult).then_inc(s_div, 1)
                    if i < NBLK:
                        je = i % ED
                        if i >= ED:
                            gpsimd.wait_ge(s_mm, i - ED + 1)  # xe WAR vs PE
                        gpsimd.wait_ge(s_exp, i + 1)
                        gpsimd.tensor_mul(xe0[je][:], x0[i % XD][:], e0[je][:])
                        gpsimd.tensor_mul(xe1[je][:], x1[i % XD][:],
                                          e1[je][:]).then_inc(s_xe, 1)
                    if i >= 2:
                        # stores for block i-2 via SWDGE queues
                        bp, hp = blk(i - 2)
                        gpsimd.wait_ge(s_ad, i - 1)
                        jo = (i - 2) % OD
                        gpsimd.dma_start(
                            out_d[bp, 0:128, hp:hp + HBLK, :],
                            o0[jo].ap().rearrange("p (h w) -> p h w", h=HBLK),
                        ).then_inc(s_st0[jo], 16)
                        gpsimd.dma_start(
                            out_d[bp, 128:256, hp:hp + HBLK, :],
                            o1[jo].ap().rearrange("p (h w) -> p h w", h=HBLK),
                        ).then_inc(s_st1[jo], 16)

            @block.tensor
            def _(tensor):
                tensor.wait_ge(s_cst, 80)
                for i in range(NBLK + 1):
                    if i < NBLK:
                        tensor.wait_ge(s_x0[i % XD], 16 * (i // XD + 1))
                        tensor.wait_ge(s_x1[i % XD], 16 * (i // XD + 1))
                        if i >= 1:
                            tensor.wait_ge(s_xpc, 2 * i)  # xp_ps WAR
                        for u in range(2):
                            for v in range(2):
                                t = 2 * u + v
                                px = bass.ts(t, 128)
                                tensor.transpose(
                                    xp_ps[u][:, v * 256:v * 256 + 128],
                                    x0[i % XD][:, px], ident[:])
                                mm = tensor.transpose(
                                    xp_ps[u][:, v * 256 + 128:v * 256 + 256],
                                    x1[i % XD][:, px], ident[:])
                                if v == 1:
                                    mm.then_inc(s_tx, 1)
                        if i >= 1:
                            tensor.wait_ge(s_snc, i)      # sn_ps WAR
                        tensor.wait_ge(s_exp, i + 1)
                        tensor.wait_ge(s_xe, i + 1)
                        for t in range(HBLK):
                            px = bass.ts(t, 128)
                            c = t * 16
                            tensor.matmul(sn_ps[:, c + 0:c + 4],
                                          e0[i % ED][:, px],
                                          mask0[:, 0:4], start=True, stop=True)
                            tensor.matmul(sn_ps[:, c + 4:c + 8],
                                          xe0[i % ED][:, px],
                                          mask0[:, 4:8], start=True, stop=True)
                            tensor.matmul(sn_ps[:, c + 8:c + 12],
                                          e1[i % ED][:, px],
                                          mask1[:, 0:4], start=True, stop=True)
                            mm = tensor.matmul(sn_ps[:, c + 12:c + 16],
                                               xe1[i % ED][:, px],
                                               mask1[:, 4:8],
                                               start=True, stop=True)
                            if t == HBLK - 1:
                                mm.then_inc(s_mm, 1)
                    if i >= 1:
                        jp = (i - 1) % 2
                        tensor.wait_ge(s_mx, i)
                        tensor.wait_ge(s_div, i)
                        if i >= 2:
                            tensor.wait_ge(s_ztc, i - 1)  # zT_ps WAR
                        for t in range(HBLK):
                            mm = tensor.transpose(
                                zT_ps[:, t * 128:(t + 1) * 128],
                                z4[jp][:, t * ZDIM:(t + 1) * ZDIM], ident[:])
                            if t == HBLK - 1:
                                mm.then_inc(s_tz, 1)
                        if i >= 3:
                            tensor.wait_ge(s_ad, i - 2)   # d_ps WAR
                        tensor.wait_ge(s_ztc, i)
                        tensor.matmul(d0_ps[jp][:], w2e0[:], zT_sb[jp][:],
                                      start=True, stop=True)
                        tensor.matmul(d1_ps[jp][:], w2e1[:], zT_sb[jp][:],
                                      start=True, stop=True).then_inc(s_dl, 1)

            @block.vector
            def _(vector):
                for i in range(NBLK + 2):
                    j = i % 2
                    if 1 <= i <= NBLK:
                        # 1/s for block i-1
                        js = (i - 1) % SD
                        if i >= SD + 1:
                            vector.wait_ge(s_div, i - SD)  # rcp WAR vs y-mul
                        vector.wait_ge(s_snc, i)
                        snp = sn_sb[js].ap().rearrange(
                            "p (t hf x g) -> p t hf x g", t=HBLK, hf=2, x=2)
                        rcv = rcp[js].ap().rearrange(
                            "p (t hf g) -> p t hf g", t=HBLK, hf=2)
                        vector.reciprocal(rcv, snp[:, :, :, 0, :]) \
                            .then_inc(s_rcp, 1)
                    if i < NBLK:
                        if i >= 2:
                            vector.wait_ge(s_tz, i - 1)   # z4 WAR vs T(z)
                        for u in range(2):
                            vector.wait_ge(s_xpc, 2 * i + u + 1)
                            for v in range(2):
                                t = 2 * u + v
                                for g in range(G):
                                    mx = vector.max(
                                        z4[j][:, t * ZDIM + 8 + 8 * g:
                                              t * ZDIM + 16 + 8 * g],
                                        xp_sb[u][j][:, v * 256 + g * 32:
                                                    v * 256 + (g + 1) * 32])
                        mx.then_inc(s_mx, 1)
                    if i >= 2:
                        jp = (i - 2) % 2
                        jo = (i - 2) % OD
                        vector.wait_ge(s_dl, i - 1)
                        if i - 2 >= OD:
                            vector.wait_ge(s_st0[jo], 16 * ((i - 2) // OD))
                            vector.wait_ge(s_st1[jo], 16 * ((i - 2) // OD))
                        vector.tensor_add(o0[jo][:], x0[(i - 2) % XD][:],
                                          d0_ps[jp][:])
                        vector.tensor_add(o1[jo][:], x1[(i - 2) % XD][:],
                                          d1_ps[jp][:]).then_inc(s_ad, 1)

    return nc


_NC_CACHE = {}


def _get_nc():
    if "nc" not in _NC_CACHE:
        _NC_CACHE["nc"] = _build_kernel()
    return _NC_CACHE["nc"]


def kernel(x, soft_w1, soft_w2, top_w1, top_w2, r, _trace=False, _tmpdir=None):
    x = np.ascontiguousarray(np.asarray(x, np.float32))
    assert x.shape == (B, C, H, W), x.shape
    consts = _build_consts(soft_w1, soft_w2, top_w1, top_w2, r)

    nc = _get_nc()
    in_maps = []
    for i in range(NCORES):
        in_maps.append({
            "x": np.ascontiguousarray(x[i * NB:(i + 1) * NB]),
            "w2eff": consts["w2eff"],
            "masks": consts["masks"],
            "ident": consts["ident"],
        })
    res = run_bass_kernel_spmd(nc, in_maps, core_ids=list(range(NCORES)),
                               trace=_trace, tmpdir=_tmpdir)
    out = np.concatenate(
        [np.asarray(res.results[i]["out"]).reshape(NB, C, H, W)
         for i in range(NCORES)], axis=0)
    if _trace:
        return out, res
    return out
